# revision 10
# baseline (speedup 1.0000x reference)
"""Trainium2 Bass kernel for AttenQueryFusion.

Computes context[:, 0:1, :] of full softmax self-attention (q=k=v=x).
Only row 0 of the attention matrix affects the output (softmax rows are
independent), so per batch b:
    q   = x[b, 0, :]                      # (D,)
    s   = x[b] @ q / sqrt(D)              # (N,)
    p   = exp(s);  Z = sum(p)             # no max-subtraction needed:
                                          # s <= ~25 -> exp fits fp32 easily
    out = (p/Z) @ x[b]                    # (D,)

Sharding: pure data parallel, batch dim 256 -> 32 per core across 8 cores.

Per-core pipeline (BC=32 batches, N=1024 tokens, D=256), per batch:
  x tile (128 partitions, t=8, d=256), token n = t*128 + p
  - q broadcast across partitions: ones(1,128)^T @ q (K=1 matmul, bf16 —
    scores are insensitive to q rounding since softmax renormalizes)
  - scores s8(128,8) via fused scalar_tensor_tensor multiply+reduce,
    split across Vector and GpSimd engines
  - exp + row-sum on Scalar engine (no max subtraction needed)
  - Z via ones matvec on PE; 1/Z on Vector; broadcast 1/Z via K=1 matmul;
    p8n = p8 * (1/Z) per-partition scalar on Vector
  - out column pair o_ps(128d,2h) = sum_t x_chunk(128n,128d)^T @ p8n_chunk
    (x-stationary fp32 matmuls, N=1 so the fp32 4x streaming penalty is
    irrelevant; cost is the 128-col weight loads)
  - staging(128, 2*BC) accumulates all batches; one PE transpose at the
    end restores d-contiguous rows, then a single 32KB DMA out.
"""

import sys

import numpy as np

sys.path.insert(0, "/opt/trn_rl_repo")

from contextlib import ExitStack

import concourse.bass as bass
import concourse.tile as tile
from concourse import bacc, mybir
from concourse.bass_utils import run_bass_kernel_spmd
from concourse.masks import make_identity

B, N, D = 256, 1024, 256
N_CORES = 8
BC = B // N_CORES  # 32 batches per core
NT = N // 128  # 8 token tiles
INV_SQRT_D = 1.0 / 16.0

# batches per DMA transfer (bigger -> better DMA efficiency)
G = 2
# how many of the 8 score chunks go to the Vector engine; the rest are done
# as GpSimd multiply + Scalar-engine accumulate (Pool can't run the fused
# scalar_tensor_tensor instruction)
N_VEC = 6

F32 = mybir.dt.float32
BF16 = mybir.dt.bfloat16
Exp = mybir.ActivationFunctionType.Exp
Copy = mybir.ActivationFunctionType.Copy


def _build_kernel():
    nc = bacc.Bacc("TRN2", target_bir_lowering=False, debug=False)
    x = nc.dram_tensor("x", [BC, N, D], F32, kind="ExternalInput").ap()
    out = nc.dram_tensor("out", [BC, 1, D], F32, kind="ExternalOutput").ap()

    with tile.TileContext(nc) as tc:
        with ExitStack() as ctx:
            _body(ctx, tc, out, x)
    nc.compile()
    return nc


def _body(ctx: ExitStack, tc: tile.TileContext, out: bass.AP, x: bass.AP):
    nc = tc.nc
    mult = mybir.AluOpType.mult

    singles = ctx.enter_context(tc.tile_pool(name="singles", bufs=1))
    xpool = ctx.enter_context(tc.tile_pool(name="xpool", bufs=3))
    spool = ctx.enter_context(tc.tile_pool(name="spool", bufs=4))
    scratch = ctx.enter_context(tc.tile_pool(name="scratch", bufs=4))
    qpool = ctx.enter_context(tc.tile_pool(name="qpool", bufs=3))
    psum_q = ctx.enter_context(tc.tile_pool(name="psum_q", bufs=2, space="PSUM"))
    psum_o = ctx.enter_context(tc.tile_pool(name="psum_o", bufs=2, space="PSUM"))
    psum_z = ctx.enter_context(tc.tile_pool(name="psum_z", bufs=2, space="PSUM"))
    psum_t = ctx.enter_context(tc.tile_pool(name="psum_t", bufs=1, space="PSUM"))

    ones_bf = singles.tile([1, 128], BF16)  # lhsT for the q broadcast matmul
    nc.vector.memset(ones_bf, 1.0)
    ones_f32 = singles.tile([1, 128], F32)  # lhsT for the zinv broadcast
    nc.vector.memset(ones_f32, 1.0)
    ones_col = singles.tile([128, 1], F32)  # rhs for the Z matvec
    nc.vector.memset(ones_col, 1.0)
    ident = singles.tile([128, 128], F32)
    make_identity(nc, ident)
    # staging[p, 2*b + h] = out[b, h*128 + p]
    staging = singles.tile([128, 2 * BC], F32)

    for g in range(BC // G):
        xt = xpool.tile([128, G, NT, D], F32, tag="xt")
        nc.sync.dma_start(
            out=xt, in_=x[g * G : (g + 1) * G].rearrange("b (t p) d -> p b t d", p=128)
        )
        for j in range(G):
            b = g * G + j
            xb = xt[:, j]  # (128, NT, D)

            # ---- q broadcast across 128 partitions (bf16 K=1 matmul) ----
            q_bf = qpool.tile([1, D], BF16, tag="q_bf")
            nc.scalar.copy(q_bf, xb[0:1, 0, :])
            qb_ps = psum_q.tile([128, D], F32, tag="qb_ps")
            nc.tensor.matmul(qb_ps, lhsT=ones_bf, rhs=q_bf, start=True, stop=True)
            qb = qpool.tile([128, D], F32, tag="qb")
            nc.scalar.copy(qb, qb_ps)

            # ---- scores: s8[p, t] = sum_d x[b, t*128+p, d] * q[d] / 16 ----
            s8 = spool.tile([128, NT], F32, tag="s8")
            for t in range(NT):
                if t < N_VEC:
                    junk = scratch.tile([128, D], F32, tag="junk_v")
                    nc.vector.scalar_tensor_tensor(
                        out=junk,
                        in0=xb[:, t, :],
                        scalar=INV_SQRT_D,
                        in1=qb,
                        op0=mult,
                        op1=mult,
                        accum_out=s8[:, t : t + 1],
                    )
                else:
                    junk_g = scratch.tile([128, D], F32, tag="junk_g")
                    nc.gpsimd.tensor_mul(junk_g, xb[:, t, :], qb)
                    junk_a = scratch.tile([128, D], F32, tag="junk_a")
                    nc.scalar.activation(
                        out=junk_a,
                        in_=junk_g,
                        func=Copy,
                        scale=INV_SQRT_D,
                        accum_out=s8[:, t : t + 1],
                    )

            # ---- p = exp(s), acc[p] = sum_t p[p, t] ----
            p8 = spool.tile([128, NT], F32, tag="p8")
            acc = spool.tile([128, 1], F32, tag="acc")
            nc.scalar.activation(out=p8, in_=s8, func=Exp, accum_out=acc)

            # ---- Z = sum_p acc; zinv = 1/Z; broadcast; p8n = p8 * zinv ----
            zz_ps = psum_z.tile([128, 2], F32, tag="zz_ps")
            nc.tensor.matmul(
                zz_ps[0:1, 0:1], lhsT=acc, rhs=ones_col, start=True, stop=True
            )
            zinv = spool.tile([1, 1], F32, tag="zinv")
            nc.vector.reciprocal(zinv, zz_ps[0:1, 0:1])
            nc.tensor.matmul(
                zz_ps[:, 1:2], lhsT=ones_f32, rhs=zinv, start=True, stop=True
            )
            p8n = spool.tile([128, NT], F32, tag="p8n")
            nc.vector.tensor_scalar_mul(p8n, p8, zz_ps[:, 1:2])

            # ---- out[d] = sum_n p8n[n] x[n, d], d = h*128 + p ----
            o_ps = psum_o.tile([128, 2], F32, tag="o_ps")
            for h in range(2):
                for t in range(NT):
                    nc.tensor.matmul(
                        o_ps[:, h : h + 1],
                        lhsT=xb[:, t, 128 * h : 128 * (h + 1)],
                        rhs=p8n[:, t : t + 1],
                        start=(t == 0),
                        stop=(t == NT - 1),
                    )
            nc.scalar.copy(staging[:, 2 * b : 2 * b + 2], o_ps)

    # ---- staging (128, 64) -> (64, 128): row (b,h) holds out[b, h*128:...] ----
    tr_ps = psum_t.tile([2 * BC, 128], F32)
    nc.tensor.transpose(tr_ps, staging, ident)
    result = singles.tile([2 * BC, 128], F32)
    nc.vector.tensor_copy(result, tr_ps)
    nc.sync.dma_start(
        out=out.rearrange("b o (h p) -> (b o h) p", p=128), in_=result
    )


_CACHED = {}


def _get_nc():
    if "nc" not in _CACHED:
        _CACHED["nc"] = _build_kernel()
    return _CACHED["nc"]


def kernel(x: np.ndarray, _trace: bool = False) -> np.ndarray:
    assert x.shape == (B, N, D) and x.dtype == np.float32, (x.shape, x.dtype)
    nc = _get_nc()
    in_maps = [
        {"x": np.ascontiguousarray(x[i * BC : (i + 1) * BC])} for i in range(N_CORES)
    ]
    res = run_bass_kernel_spmd(nc, in_maps, core_ids=list(range(N_CORES)), trace=_trace)
    _CACHED["last_results"] = res
    return np.concatenate([r["out"] for r in res.results], axis=0)


if __name__ == "__main__":
    rng = np.random.default_rng(0)
    x = rng.standard_normal((B, N, D), dtype=np.float32)
    y = kernel(x)
    print("kernel output", y.shape, y.dtype)
